# revision 34
# baseline (speedup 1.0000x reference)
"""Cross-attention kernel for one TRN2 chip (8 NeuronCores).

Sharding: core = (batch b in {0,1}) x (head-group of 4 heads).  Each core
computes attention for its 4 heads of its batch element and a partial output
projection [N, 1024] in bf16; the host sums the 4 partials per batch and
adds the bias in f32.

Host-side prep: x/context are transposed and cast to bf16 on the host
(xT/cT [C, seq]) so the kernel needs no on-chip transposes; weights are
pre-sliced per head-group and cast to bf16.  (fp8 was tried for the AV
matmuls and rejected: attention output is a near-zero-mean average of V,
so per-element quantization noise is amplified ~45x in relative terms —
fp8 P/V measured 3.3% rel err vs the 2% budget.)

Per-core schedule (all matmuls bf16, fp32 PSUM):
  loads    cT on both HWDGE queues first (V-proj gates on it), xT behind it
           plus the SWDGE queue; weights lead on SWDGE
  proj     cc-outer streaming (4 PSUM accumulators, stationary reused 4x):
           K(dc0) chases the cT chunks, then V, then Q(dc0) chases xT.
           K(dc1)/Q(dc1) run entirely as micro-steps inside attention.
  attn     8 passes (pf outer, dc inner); per m-tile: two QK matmuls
           (row-group-paired heads, concurrent on the PE) into one
           [128, 2x512] PSUM tile, ONE [128,1024] exp on ScalarE (halves
           the 352-cycle/instr overhead), two AV matmuls lagging one
           m-tile.  V carries a ones column so AV also accumulates the
           softmax denominators.  A step scheduler drains deferred work
           (dc1 proj chunks, epilogue pieces, y half-chunks) <=2 steps per
           m-round; oT copies run as urgent steps at round 0.
  epilogue sums row -> DMA-scatter to [128,4] -> DVE reciprocal ->
           DMA-gather to bf16 @partition0 -> ones-matmul broadcast on PE ->
           one DVE multiply (odd head lands via a partition-shift DMA).
PSUM budget: sT 2x2 banks + oT 2x1 + scratch 2x1 = 8 banks.
"""

from collections import deque
from functools import partial

import numpy as np
import ml_dtypes

import concourse.bass as bass
import concourse.mybir as mybir
import concourse.tile as tile
from concourse import bacc
from concourse.bass import ts
from concourse.bass_utils import run_bass_kernel_spmd
from concourse.masks import make_identity

B, N, M, C = 2, 2048, 2048, 1024
HEADS, DH = 16, 64
H_PER = 4                # heads per core
DHC = H_PER * DH         # 256: per-core slice of INNER
SCALE = DH ** -0.5
P = 128
FD = 512                 # matmul moving free dim
NCH = N // FD            # 4 n-quarters
MT = M // P              # 16 m-tiles
CCH = C // P             # 8 contraction chunks
N_CORES = 8

F32 = mybir.dt.float32
BF16 = mybir.dt.bfloat16
EXP = mybir.ActivationFunctionType.Exp
BF = ml_dtypes.bfloat16

_CACHE = {}


def _build():
    nc = bacc.Bacc("TRN2", target_bir_lowering=False, debug=False,
                   num_devices=N_CORES, num_swdge_queues=4)

    xT_d = nc.dram_tensor("xT", (C, N), BF16, kind="ExternalInput").ap()
    cT_d = nc.dram_tensor("cT", (C, M), BF16, kind="ExternalInput").ap()
    msk_d = nc.dram_tensor("msk", (M, 1), F32, kind="ExternalInput").ap()
    wq_d = nc.dram_tensor("wq", (C, DHC), BF16, kind="ExternalInput").ap()
    wk_d = nc.dram_tensor("wk", (C, DHC), BF16, kind="ExternalInput").ap()
    wv_d = nc.dram_tensor("wv", (C, DHC), BF16, kind="ExternalInput").ap()
    wo_d = nc.dram_tensor("wo", (DHC, C), BF16, kind="ExternalInput").ap()
    y_d = nc.dram_tensor("y", (N, C), BF16, kind="ExternalOutput").ap()

    with tile.TileContext(nc) as tc:
        with (
            tc.tile_pool(name="const", bufs=1) as const,
            tc.tile_pool(name="pTp", bufs=3) as pTp,
            tc.tile_pool(name="ofp", bufs=10) as ofp,
            tc.tile_pool(name="recp", bufs=2) as recp,
            tc.tile_pool(name="stg", bufs=2) as stgp,
            tc.tile_pool(name="yp", bufs=3) as yp,
        ):
            # ---- persistent SBUF tensors ----
            xT = const.tile([P, CCH, N], BF16, name="xT")
            cT = const.tile([P, CCH, M], BF16, name="cT")
            qT = [const.tile([P, N], BF16, name=f"qT{dc}") for dc in range(2)]
            kT = [const.tile([P, M], BF16, name=f"kT{dc}") for dc in range(2)]
            oTp = [const.tile([P, N], BF16, name=f"oTp{dc}") for dc in range(2)]
            v_sb = const.tile([P, MT, H_PER, DH + 1], BF16, name="v")
            wq_sb = const.tile([P, CCH, DHC], BF16, name="wq")
            wk_sb = const.tile([P, CCH, DHC], BF16, name="wk")
            wv_sb = const.tile([P, CCH, DHC], BF16, name="wv")
            wo_sb = const.tile([P, 2, C], BF16, name="wo")
            msk_sb = const.tile([P, MT, 1], F32, name="msk")
            ones_sb = const.tile([1, DH], BF16, name="ones")

            # ~3.3MB per queue; wk leads sync chunk-by-chunk (the first
            # K-proj matmul only needs wk[cc0] + cT[cc0]),
            # cT before xT everywhere (V gates on full cT).
            wk_r = wk_d.rearrange("(cc p) d -> p cc d", p=P)
            for cc in range(CCH):
                nc.sync.dma_start(out=wk_sb[:, cc], in_=wk_r[:, cc])
            nc.gpsimd.dma_start(
                out=wv_sb, in_=wv_d.rearrange("(cc p) d -> p cc d", p=P))
            nc.gpsimd.dma_start(
                out=wq_sb, in_=wq_d.rearrange("(cc p) d -> p cc d", p=P))
            nc.gpsimd.dma_start(
                out=msk_sb, in_=msk_d.rearrange("(t p) o -> p t o", p=P))
            cT_r = cT_d.rearrange("(cc p) m -> p cc m", p=P)
            xT_r = xT_d.rearrange("(cc p) n -> p cc n", p=P)
            for cc in range(CCH):
                (nc.scalar if cc % 2 == 0 else nc.sync).dma_start(
                    out=cT[:, cc], in_=cT_r[:, cc])
            for cc, eng in zip(range(CCH), (nc.scalar, nc.sync, nc.scalar,
                                            nc.sync, nc.gpsimd, nc.gpsimd,
                                            nc.gpsimd, nc.gpsimd)):
                eng.dma_start(out=xT[:, cc], in_=xT_r[:, cc])
            nc.gpsimd.dma_start(
                out=wo_sb, in_=wo_d.rearrange("(dc p) e -> p dc e", p=P))
            nc.vector.memset(ones_sb, 1.0)
            nc.vector.memset(v_sb[:, :, :, DH:DH + 1], 1.0)

            cp = [0]

            def copy_alt(dst, src):
                if cp[0] % 2 == 0:
                    nc.vector.tensor_copy(dst, src)
                else:
                    nc.scalar.copy(dst, src)
                cp[0] += 1

            # ---- proj phase: K(both dc, interleaved so dc1 hides in the
            # cT chunk-arrival stalls), V, Q(dc0); cc-outer streaming ----
            ppk_cm = tc.tile_pool(name="ps_ppk", bufs=1, space="PSUM")
            ppk = ppk_cm.__enter__()
            pssk = [ppk.tile([P, FD], F32, name=f"pk{dc}{j}",
                             tag=f"pk{dc}{j}")
                    for dc in range(2) for j in range(NCH)]
            for cc in range(CCH):
                for dc in range(2):
                    for j in range(NCH):
                        nc.tensor.matmul(
                            pssk[dc * NCH + j],
                            lhsT=wk_sb[:, cc, ts(dc, P)],
                            rhs=cT[:, cc, ts(j, FD)],
                            start=(cc == 0), stop=(cc == CCH - 1))
            for dc in range(2):
                for j in range(NCH):
                    copy_alt(kT[dc][:, ts(j, FD)], pssk[dc * NCH + j])
            ppk_cm.__exit__(None, None, None)

            pp_cm = tc.tile_pool(name="ps_pp", bufs=1, space="PSUM")
            pp = pp_cm.__enter__()

            def proj_stream(w_sb, srcT, dc, dstT):
                pss = [pp.tile([P, FD], F32, name=f"pj{j}", tag=f"pj{j}")
                       for j in range(NCH)]
                for cc in range(CCH):
                    for j in range(NCH):
                        nc.tensor.matmul(
                            pss[j], lhsT=w_sb[:, cc, ts(dc, P)],
                            rhs=srcT[:, cc, ts(j, FD)],
                            start=(cc == 0), stop=(cc == CCH - 1))
                for j in range(NCH):
                    copy_alt(dstT[:, ts(j, FD)], pss[j])

            for m in range(MT):
                vp = pp.tile([P, DHC], F32, name="vp", tag="vp", bufs=3)
                for cc in range(CCH):
                    nc.tensor.matmul(
                        vp, lhsT=cT[:, cc, ts(m, P)], rhs=wv_sb[:, cc, :],
                        start=(cc == 0), stop=(cc == CCH - 1))
                # copy + mask in one DVE op; ones column was memset upfront
                nc.vector.tensor_scalar_mul(
                    v_sb[:, m, :, 0:DH],
                    vp.rearrange("p (h d) -> p h d", h=H_PER),
                    msk_sb[:, m, :])
                nc.vector.tensor_scalar_mul(
                    v_sb[:, m, :, DH:DH + 1], v_sb[:, m, :, DH:DH + 1],
                    msk_sb[:, m, :])
            # only Q(dc0) chunk j0 up front (pass (0,0) reads it at round 0);
            # the rest stream as in-attention steps
            psq = pp.tile([P, FD], F32, name="pj0", tag="pj0")
            for cc in range(CCH):
                nc.tensor.matmul(
                    psq, lhsT=wq_sb[:, cc, ts(0, P)],
                    rhs=xT[:, cc, ts(0, FD)],
                    start=(cc == 0), stop=(cc == CCH - 1))
            copy_alt(qT[0][:, ts(0, FD)], psq)
            pp_cm.__exit__(None, None, None)

            # ---- attention ----
            ps_s_cm = tc.tile_pool(name="ps_s", bufs=2, space="PSUM")
            ps_s = ps_s_cm.__enter__()
            ps_o_cm = tc.tile_pool(name="ps_o", bufs=1, space="PSUM")
            ps_o = ps_o_cm.__enter__()
            ps_x_cm = tc.tile_pool(name="ps_x", bufs=2, space="PSUM")
            ps_x = ps_x_cm.__enter__()

            def epilogue_copy(oT, dst):
                for s in range(2):
                    o_f = ofp.tile([DH + 1, FD], F32, name="o_f")
                    nc.vector.tensor_copy(o_f, oT[s])
                    dst.append(o_f)

            def epilogue_fin(pf, dc, s, o_fs):
                o_f = o_fs[s]
                # reciprocal of the 1-partition sums row is serial on DVE;
                # scatter across 128 partitions (4 elems/lane), recip,
                # gather back to bf16 at partition 0.
                srec = recp.tile([P, FD // P], F32, name="srec", tag="srec")
                nc.gpsimd.dma_start(out=srec, in_=o_f[DH:DH + 1, :])
                rb = recp.tile([P, FD // P], BF16, name="rb", tag="rb")
                with nc.allow_low_precision("softmax denom reciprocal"):
                    nc.vector.reciprocal(rb, srec)
                rec = recp.tile([1, FD], BF16, name="rec")
                nc.gpsimd.dma_start(out=rec, in_=rb)
                bc = ps_x.tile([DH, FD], F32, name="bc", tag="scr")
                nc.tensor.matmul(bc, lhsT=ones_sb, rhs=rec,
                                 start=True, stop=True)
                if s == 0:
                    nc.vector.tensor_mul(
                        oTp[dc][0:DH, ts(pf, FD)], o_f[0:DH, :], bc)
                else:
                    st = stgp.tile([DH, FD], BF16, name="st")
                    nc.vector.tensor_mul(st, o_f[0:DH, :], bc)
                    nc.gpsimd.dma_start(
                        out=oTp[dc][DH:P, ts(pf, FD)], in_=st)

            def y_quarter(i, col, dc, y_sbs):
                # one matmul per step so a step never outruns the exp pace
                if (col, dc) == (0, 0):
                    y_sbs[i] = (yp.tile([P, C], BF16, name="ysb"), {})
                y_sb, y_pss = y_sbs[i]
                if dc == 0:
                    y_pss[col] = ps_x.tile([P, FD], F32, name="y_ps",
                                           tag="scr")
                nc.tensor.matmul(
                    y_pss[col], lhsT=oTp[dc][:, ts(i, P)],
                    rhs=wo_sb[:, dc, ts(col, FD)],
                    start=(dc == 0), stop=(dc == 1))
                if dc == 1:
                    nc.vector.tensor_copy(y_sb[:, ts(col, FD)], y_pss[col])
                    del y_pss[col]
                    if col == 1:
                        nc.sync.dma_start(out=y_d[ts(i, P), :], in_=y_sb)
                        del y_sbs[i]

            def proj_late_steps(w_sb, srcT, dc, j, dstT):
                # one K/Q chunk for dc1 as 9 micro-steps (8 MMs + copy),
                # accumulating in a held scratch-tag PSUM slot
                state = {}

                def mm(cc):
                    if cc == 0:
                        state["ps"] = ps_x.tile([P, FD], F32, name="plp",
                                                tag="scr")
                    nc.tensor.matmul(
                        state["ps"], lhsT=w_sb[:, cc, ts(dc, P)],
                        rhs=srcT[:, cc, ts(j, FD)],
                        start=(cc == 0), stop=(cc == CCH - 1))

                def fin():
                    # DVE only: ScalarE is exp-saturated during attention
                    nc.vector.tensor_copy(dstT[:, ts(j, FD)], state["ps"])

                return [partial(mm, cc) for cc in range(CCH)] + [fin]

            def attention_pass(pf, dc, steps, urgent):
                oT = [ps_o.tile([DH + 1, FD], F32, name=f"oT{s}",
                                tag=f"oT{s}") for s in range(2)]
                pts = {}
                prev = None
                for m in range(MT):
                    while urgent:
                        urgent.popleft()()
                    sT = ps_s.tile([P, 2, FD], F32, name="sT", tag="sT")
                    for s in range(2):
                        nc.tensor.matmul(
                            sT[:, s, :],
                            lhsT=kT[dc][ts(s, DH), ts(m, P)],
                            rhs=qT[dc][ts(s, DH), ts(pf, FD)],
                            start=True, stop=True)
                    pT = pTp.tile([P, 2, FD], BF16, name="pT")
                    nc.scalar.activation(pT, sT, EXP, scale=SCALE)
                    pts[m] = pT
                    # deferred work AFTER the QK/exp emission: a step's
                    # matmul then fills the exp latency instead of delaying
                    # the exp chain
                    if steps:
                        steps.popleft()()
                    if prev is not None:
                        for s in range(2):
                            nc.tensor.matmul(
                                oT[s], lhsT=v_sb[:, prev, 2 * dc + s, :],
                                rhs=pts[prev][:, s, :],
                                start=(prev == 0), stop=(prev == MT - 1))
                        del pts[prev]
                    prev = m
                for s in range(2):
                    nc.tensor.matmul(
                        oT[s], lhsT=v_sb[:, prev, 2 * dc + s, :],
                        rhs=pts[prev][:, s, :],
                        start=(prev == 0), stop=(prev == MT - 1))
                return oT

            # Remaining Q chunks as in-attention steps, interleaved in order
            # of first consumption: Q1 j by pass (j,1) = pass 2j+2, Q0 j by
            # pass (j,0) = pass 2j+1.
            steps = deque()
            for dcj in ((1, 0), (0, 1), (1, 1), (0, 2), (1, 2), (0, 3),
                        (1, 3)):
                dc, jj = dcj
                steps.extend(proj_late_steps(wq_sb, xT, dc, jj, qT[dc]))

            urgent = deque()
            y_sbs = {}
            for pf in range(NCH):
                for dc in range(2):
                    oT = attention_pass(pf, dc, steps, urgent)
                    o_fs = []
                    urgent.append(partial(epilogue_copy, oT, o_fs))
                    steps.append(partial(epilogue_fin, pf, dc, 0, o_fs))
                    steps.append(partial(epilogue_fin, pf, dc, 1, o_fs))
                    if dc == 1:
                        for i in range(4 * pf, 4 * pf + 4):
                            for col in range(2):
                                for ydc in range(2):
                                    steps.append(partial(
                                        y_quarter, i, col, ydc, y_sbs))
            while urgent:
                urgent.popleft()()
            while steps:
                steps.popleft()()

            ps_x_cm.__exit__(None, None, None)
            ps_o_cm.__exit__(None, None, None)
            ps_s_cm.__exit__(None, None, None)

    nc.compile()
    return nc


def _in_maps(x, context, mask, Wq, Wk, Wv, Wo):
    xTb = [np.ascontiguousarray(
        np.asarray(x[b], dtype=np.float32).T.astype(BF)) for b in range(B)]
    cTb = [np.ascontiguousarray(
        np.asarray(context[b], dtype=np.float32).T.astype(BF))
        for b in range(B)]
    mb = [np.ascontiguousarray(
        np.asarray(mask[b], dtype=np.float32).reshape(M, 1))
        for b in range(B)]
    wq_h, wk_h, wv_h, wo_h = [], [], [], []
    for hg in range(H_PER):
        c0 = hg * DHC
        wq_h.append(np.ascontiguousarray(
            np.asarray(Wq[:, c0:c0 + DHC], np.float32).astype(BF)))
        wk_h.append(np.ascontiguousarray(
            np.asarray(Wk[:, c0:c0 + DHC], np.float32).astype(BF)))
        wv_h.append(np.ascontiguousarray(
            np.asarray(Wv[:, c0:c0 + DHC], np.float32).astype(BF)))
        wo_h.append(np.ascontiguousarray(
            np.asarray(Wo[c0:c0 + DHC, :], np.float32).astype(BF)))
    maps = []
    for core in range(N_CORES):
        b, hg = core // H_PER, core % H_PER
        maps.append({
            "xT": xTb[b], "cT": cTb[b], "msk": mb[b],
            "wq": wq_h[hg], "wk": wk_h[hg], "wv": wv_h[hg], "wo": wo_h[hg],
        })
    return maps


def _gather(results, bo):
    out = np.zeros((B, N, C), dtype=np.float32)
    for core in range(N_CORES):
        out[core // H_PER] += np.asarray(results[core]["y"],
                                         dtype=np.float32)
    out += np.asarray(bo, dtype=np.float32)
    return out


def kernel(x, context, mask, Wq, Wk, Wv, Wo, bo, **extra_kwargs):
    if "nc" not in _CACHE:
        _CACHE["nc"] = _build()
    nc = _CACHE["nc"]
    maps = _in_maps(x, context, mask, Wq, Wk, Wv, Wo)
    res = run_bass_kernel_spmd(nc, maps, core_ids=list(range(N_CORES)),
                               **extra_kwargs)
    out = _gather(res.results, bo)
    if extra_kwargs:
        _CACHE["last_result"] = res
    return out


# revision 35
# speedup vs baseline: 1.0004x; 1.0004x over previous
"""Cross-attention kernel for one TRN2 chip (8 NeuronCores).

Sharding: core = (batch b in {0,1}) x (head-group of 4 heads).  Each core
computes attention for its 4 heads of its batch element and a partial output
projection [N, 1024] in bf16; the host sums the 4 partials per batch and
adds the bias in f32.

Host-side prep: x/context are transposed and cast to bf16 on the host
(xT/cT [C, seq]) so the kernel needs no on-chip transposes; weights are
pre-sliced per head-group and cast to bf16.  (fp8 was tried for the AV
matmuls and rejected: attention output is a near-zero-mean average of V,
so per-element quantization noise is amplified ~45x in relative terms —
fp8 P/V measured 3.3% rel err vs the 2% budget.)

Per-core schedule (all matmuls bf16, fp32 PSUM):
  loads    cT on both HWDGE queues first (V-proj gates on it), xT behind it
           plus the SWDGE queue; weights lead on SWDGE
  proj     cc-outer streaming (4 PSUM accumulators, stationary reused 4x):
           K(dc0) chases the cT chunks, then V, then Q(dc0) chases xT.
           K(dc1)/Q(dc1) run entirely as micro-steps inside attention.
  attn     8 passes (pf outer, dc inner); per m-tile: two QK matmuls
           (row-group-paired heads, concurrent on the PE) into one
           [128, 2x512] PSUM tile, ONE [128,1024] exp on ScalarE (halves
           the 352-cycle/instr overhead), two AV matmuls lagging one
           m-tile.  V carries a ones column so AV also accumulates the
           softmax denominators.  A step scheduler drains deferred work
           (dc1 proj chunks, epilogue pieces, y half-chunks) <=2 steps per
           m-round; oT copies run as urgent steps at round 0.
  epilogue sums row -> DMA-scatter to [128,4] -> DVE reciprocal ->
           DMA-gather to bf16 @partition0 -> ones-matmul broadcast on PE ->
           one DVE multiply (odd head lands via a partition-shift DMA).
PSUM budget: sT 2x2 banks + oT 2x1 + scratch 2x1 = 8 banks.
"""

from collections import deque
from functools import partial

import numpy as np
import ml_dtypes

import concourse.bass as bass
import concourse.mybir as mybir
import concourse.tile as tile
from concourse import bacc
from concourse.bass import ts
from concourse.bass_utils import run_bass_kernel_spmd

B, N, M, C = 2, 2048, 2048, 1024
HEADS, DH = 16, 64
H_PER = 4                # heads per core
DHC = H_PER * DH         # 256: per-core slice of INNER
SCALE = DH ** -0.5
P = 128
FD = 512                 # matmul moving free dim
NCH = N // FD            # 4 n-quarters
MT = M // P              # 16 m-tiles
CCH = C // P             # 8 contraction chunks
N_CORES = 8

F32 = mybir.dt.float32
BF16 = mybir.dt.bfloat16
EXP = mybir.ActivationFunctionType.Exp
BF = ml_dtypes.bfloat16

_CACHE = {}


def _build():
    nc = bacc.Bacc("TRN2", target_bir_lowering=False, debug=False,
                   num_devices=N_CORES, num_swdge_queues=4)

    xT_d = nc.dram_tensor("xT", (C, N), BF16, kind="ExternalInput").ap()
    cT_d = nc.dram_tensor("cT", (C, M), BF16, kind="ExternalInput").ap()
    msk_d = nc.dram_tensor("msk", (M, 1), F32, kind="ExternalInput").ap()
    wq_d = nc.dram_tensor("wq", (C, DHC), BF16, kind="ExternalInput").ap()
    wk_d = nc.dram_tensor("wk", (C, DHC), BF16, kind="ExternalInput").ap()
    wv_d = nc.dram_tensor("wv", (C, DHC), BF16, kind="ExternalInput").ap()
    wo_d = nc.dram_tensor("wo", (DHC, C), BF16, kind="ExternalInput").ap()
    y_d = nc.dram_tensor("y", (N, C), BF16, kind="ExternalOutput").ap()

    with tile.TileContext(nc) as tc:
        with (
            tc.tile_pool(name="const", bufs=1) as const,
            tc.tile_pool(name="pTp", bufs=3) as pTp,
            tc.tile_pool(name="ofp", bufs=10) as ofp,
            tc.tile_pool(name="recp", bufs=2) as recp,
            tc.tile_pool(name="stg", bufs=2) as stgp,
            tc.tile_pool(name="yp", bufs=3) as yp,
        ):
            # ---- persistent SBUF tensors ----
            xT = const.tile([P, CCH, N], BF16, name="xT")
            cT = const.tile([P, CCH, M], BF16, name="cT")
            qT = [const.tile([P, N], BF16, name=f"qT{dc}") for dc in range(2)]
            kT = [const.tile([P, M], BF16, name=f"kT{dc}") for dc in range(2)]
            oTp = [const.tile([P, N], BF16, name=f"oTp{dc}") for dc in range(2)]
            v_sb = const.tile([P, MT, H_PER, DH + 1], BF16, name="v")
            wq_sb = const.tile([P, CCH, DHC], BF16, name="wq")
            wk_sb = const.tile([P, CCH, DHC], BF16, name="wk")
            wv_sb = const.tile([P, CCH, DHC], BF16, name="wv")
            wo_sb = const.tile([P, 2, C], BF16, name="wo")
            msk_sb = const.tile([P, MT, 1], F32, name="msk")
            ones_sb = const.tile([1, DH], BF16, name="ones")

            # ~3.3MB per queue; wk leads sync chunk-by-chunk (the first
            # K-proj matmul only needs wk[cc0] + cT[cc0]),
            # cT before xT everywhere (V gates on full cT).
            wk_r = wk_d.rearrange("(cc p) d -> p cc d", p=P)
            for cc in range(CCH):
                nc.sync.dma_start(out=wk_sb[:, cc], in_=wk_r[:, cc])
            nc.gpsimd.dma_start(
                out=wv_sb, in_=wv_d.rearrange("(cc p) d -> p cc d", p=P))
            nc.gpsimd.dma_start(
                out=wq_sb, in_=wq_d.rearrange("(cc p) d -> p cc d", p=P))
            nc.gpsimd.dma_start(
                out=msk_sb, in_=msk_d.rearrange("(t p) o -> p t o", p=P))
            cT_r = cT_d.rearrange("(cc p) m -> p cc m", p=P)
            xT_r = xT_d.rearrange("(cc p) n -> p cc n", p=P)
            for cc in range(CCH):
                (nc.scalar if cc % 2 == 0 else nc.sync).dma_start(
                    out=cT[:, cc], in_=cT_r[:, cc])
            for cc, eng in zip(range(CCH), (nc.scalar, nc.sync, nc.scalar,
                                            nc.sync, nc.gpsimd, nc.gpsimd,
                                            nc.gpsimd, nc.gpsimd)):
                eng.dma_start(out=xT[:, cc], in_=xT_r[:, cc])
            nc.gpsimd.dma_start(
                out=wo_sb, in_=wo_d.rearrange("(dc p) e -> p dc e", p=P))
            nc.vector.memset(ones_sb, 1.0)
            nc.vector.memset(v_sb[:, :, :, DH:DH + 1], 1.0)

            cp = [0]

            def copy_alt(dst, src):
                if cp[0] % 2 == 0:
                    nc.vector.tensor_copy(dst, src)
                else:
                    nc.scalar.copy(dst, src)
                cp[0] += 1

            # ---- proj phase: K(both dc, interleaved so dc1 hides in the
            # cT chunk-arrival stalls), V, Q(dc0); cc-outer streaming ----
            ppk_cm = tc.tile_pool(name="ps_ppk", bufs=1, space="PSUM")
            ppk = ppk_cm.__enter__()
            pssk = [ppk.tile([P, FD], F32, name=f"pk{dc}{j}",
                             tag=f"pk{dc}{j}")
                    for dc in range(2) for j in range(NCH)]
            for cc in range(CCH):
                for dc in range(2):
                    for j in range(NCH):
                        nc.tensor.matmul(
                            pssk[dc * NCH + j],
                            lhsT=wk_sb[:, cc, ts(dc, P)],
                            rhs=cT[:, cc, ts(j, FD)],
                            start=(cc == 0), stop=(cc == CCH - 1))
            for dc in range(2):
                for j in range(NCH):
                    copy_alt(kT[dc][:, ts(j, FD)], pssk[dc * NCH + j])
            ppk_cm.__exit__(None, None, None)

            pp_cm = tc.tile_pool(name="ps_pp", bufs=1, space="PSUM")
            pp = pp_cm.__enter__()

            def proj_stream(w_sb, srcT, dc, dstT):
                pss = [pp.tile([P, FD], F32, name=f"pj{j}", tag=f"pj{j}")
                       for j in range(NCH)]
                for cc in range(CCH):
                    for j in range(NCH):
                        nc.tensor.matmul(
                            pss[j], lhsT=w_sb[:, cc, ts(dc, P)],
                            rhs=srcT[:, cc, ts(j, FD)],
                            start=(cc == 0), stop=(cc == CCH - 1))
                for j in range(NCH):
                    copy_alt(dstT[:, ts(j, FD)], pss[j])

            for m in range(MT):
                vp = pp.tile([P, DHC], F32, name="vp", tag="vp", bufs=3)
                for cc in range(CCH):
                    nc.tensor.matmul(
                        vp, lhsT=cT[:, cc, ts(m, P)], rhs=wv_sb[:, cc, :],
                        start=(cc == 0), stop=(cc == CCH - 1))
                # copy + mask in one DVE op; ones column was memset upfront
                nc.vector.tensor_scalar_mul(
                    v_sb[:, m, :, 0:DH],
                    vp.rearrange("p (h d) -> p h d", h=H_PER),
                    msk_sb[:, m, :])
                nc.vector.tensor_scalar_mul(
                    v_sb[:, m, :, DH:DH + 1], v_sb[:, m, :, DH:DH + 1],
                    msk_sb[:, m, :])
            # only Q(dc0) chunk j0 up front (pass (0,0) reads it at round 0);
            # the rest stream as in-attention steps
            psq = pp.tile([P, FD], F32, name="pj0", tag="pj0")
            for cc in range(CCH):
                nc.tensor.matmul(
                    psq, lhsT=wq_sb[:, cc, ts(0, P)],
                    rhs=xT[:, cc, ts(0, FD)],
                    start=(cc == 0), stop=(cc == CCH - 1))
            copy_alt(qT[0][:, ts(0, FD)], psq)
            pp_cm.__exit__(None, None, None)

            # ---- attention ----
            ps_s_cm = tc.tile_pool(name="ps_s", bufs=2, space="PSUM")
            ps_s = ps_s_cm.__enter__()
            ps_o_cm = tc.tile_pool(name="ps_o", bufs=1, space="PSUM")
            ps_o = ps_o_cm.__enter__()
            ps_x_cm = tc.tile_pool(name="ps_x", bufs=2, space="PSUM")
            ps_x = ps_x_cm.__enter__()

            def epilogue_copy(oT, dst):
                for s in range(2):
                    o_f = ofp.tile([DH + 1, FD], F32, name="o_f")
                    nc.vector.tensor_copy(o_f, oT[s])
                    dst.append(o_f)

            def epilogue_fin(pf, dc, s, o_fs):
                o_f = o_fs[s]
                # reciprocal of the 1-partition sums row is serial on DVE;
                # scatter across 128 partitions (4 elems/lane), recip,
                # gather back to bf16 at partition 0.
                srec = recp.tile([P, FD // P], F32, name="srec", tag="srec")
                nc.gpsimd.dma_start(out=srec, in_=o_f[DH:DH + 1, :])
                rb = recp.tile([P, FD // P], BF16, name="rb", tag="rb")
                with nc.allow_low_precision("softmax denom reciprocal"):
                    nc.vector.reciprocal(rb, srec)
                rec = recp.tile([1, FD], BF16, name="rec")
                nc.gpsimd.dma_start(out=rec, in_=rb)
                bc = ps_x.tile([DH, FD], F32, name="bc", tag="scr")
                nc.tensor.matmul(bc, lhsT=ones_sb, rhs=rec,
                                 start=True, stop=True)
                if s == 0:
                    nc.vector.tensor_mul(
                        oTp[dc][0:DH, ts(pf, FD)], o_f[0:DH, :], bc)
                else:
                    st = stgp.tile([DH, FD], BF16, name="st")
                    nc.vector.tensor_mul(st, o_f[0:DH, :], bc)
                    nc.gpsimd.dma_start(
                        out=oTp[dc][DH:P, ts(pf, FD)], in_=st)

            def y_quarter(i, col, dc, y_sbs):
                # one matmul per step so a step never outruns the exp pace
                if (col, dc) == (0, 0):
                    y_sbs[i] = (yp.tile([P, C], BF16, name="ysb"), {})
                y_sb, y_pss = y_sbs[i]
                if dc == 0:
                    y_pss[col] = ps_x.tile([P, FD], F32, name="y_ps",
                                           tag="scr")
                nc.tensor.matmul(
                    y_pss[col], lhsT=oTp[dc][:, ts(i, P)],
                    rhs=wo_sb[:, dc, ts(col, FD)],
                    start=(dc == 0), stop=(dc == 1))
                if dc == 1:
                    nc.vector.tensor_copy(y_sb[:, ts(col, FD)], y_pss[col])
                    del y_pss[col]
                    if col == 1:
                        nc.sync.dma_start(out=y_d[ts(i, P), :], in_=y_sb)
                        del y_sbs[i]

            def proj_late_steps(w_sb, srcT, dc, j, dstT):
                # one K/Q chunk for dc1 as 9 micro-steps (8 MMs + copy),
                # accumulating in a held scratch-tag PSUM slot
                state = {}

                def mm(cc):
                    if cc == 0:
                        state["ps"] = ps_x.tile([P, FD], F32, name="plp",
                                                tag="scr")
                    nc.tensor.matmul(
                        state["ps"], lhsT=w_sb[:, cc, ts(dc, P)],
                        rhs=srcT[:, cc, ts(j, FD)],
                        start=(cc == 0), stop=(cc == CCH - 1))

                def fin():
                    # DVE only: ScalarE is exp-saturated during attention
                    nc.vector.tensor_copy(dstT[:, ts(j, FD)], state["ps"])

                return [partial(mm, cc) for cc in range(CCH)] + [fin]

            def attention_pass(pf, dc, steps, urgent):
                oT = [ps_o.tile([DH + 1, FD], F32, name=f"oT{s}",
                                tag=f"oT{s}") for s in range(2)]
                pts = {}
                prev = None
                for m in range(MT):
                    while urgent:
                        urgent.popleft()()
                    sT = ps_s.tile([P, 2, FD], F32, name="sT", tag="sT")
                    for s in range(2):
                        nc.tensor.matmul(
                            sT[:, s, :],
                            lhsT=kT[dc][ts(s, DH), ts(m, P)],
                            rhs=qT[dc][ts(s, DH), ts(pf, FD)],
                            start=True, stop=True)
                    pT = pTp.tile([P, 2, FD], BF16, name="pT")
                    nc.scalar.activation(pT, sT, EXP, scale=SCALE)
                    pts[m] = pT
                    # deferred work AFTER the QK/exp emission: a step's
                    # matmul then fills the exp latency instead of delaying
                    # the exp chain
                    if steps:
                        steps.popleft()()
                    if prev is not None:
                        for s in range(2):
                            nc.tensor.matmul(
                                oT[s], lhsT=v_sb[:, prev, 2 * dc + s, :],
                                rhs=pts[prev][:, s, :],
                                start=(prev == 0), stop=(prev == MT - 1))
                        del pts[prev]
                    prev = m
                for s in range(2):
                    nc.tensor.matmul(
                        oT[s], lhsT=v_sb[:, prev, 2 * dc + s, :],
                        rhs=pts[prev][:, s, :],
                        start=(prev == 0), stop=(prev == MT - 1))
                return oT

            # Remaining Q chunks as in-attention steps, interleaved in order
            # of first consumption: Q1 j by pass (j,1) = pass 2j+2, Q0 j by
            # pass (j,0) = pass 2j+1.
            steps = deque()
            for dcj in ((1, 0), (0, 1), (1, 1), (0, 2), (1, 2), (0, 3),
                        (1, 3)):
                dc, jj = dcj
                steps.extend(proj_late_steps(wq_sb, xT, dc, jj, qT[dc]))

            urgent = deque()
            y_sbs = {}
            for pf in range(NCH):
                for dc in range(2):
                    oT = attention_pass(pf, dc, steps, urgent)
                    o_fs = []
                    urgent.append(partial(epilogue_copy, oT, o_fs))
                    steps.append(partial(epilogue_fin, pf, dc, 0, o_fs))
                    steps.append(partial(epilogue_fin, pf, dc, 1, o_fs))
                    if dc == 1:
                        for i in range(4 * pf, 4 * pf + 4):
                            for col in range(2):
                                for ydc in range(2):
                                    steps.append(partial(
                                        y_quarter, i, col, ydc, y_sbs))
            while urgent:
                urgent.popleft()()
            while steps:
                steps.popleft()()

            ps_x_cm.__exit__(None, None, None)
            ps_o_cm.__exit__(None, None, None)
            ps_s_cm.__exit__(None, None, None)

    nc.compile()
    return nc


def _in_maps(x, context, mask, Wq, Wk, Wv, Wo):
    xTb = [np.ascontiguousarray(
        np.asarray(x[b], dtype=np.float32).T.astype(BF)) for b in range(B)]
    cTb = [np.ascontiguousarray(
        np.asarray(context[b], dtype=np.float32).T.astype(BF))
        for b in range(B)]
    mb = [np.ascontiguousarray(
        np.asarray(mask[b], dtype=np.float32).reshape(M, 1))
        for b in range(B)]
    wq_h, wk_h, wv_h, wo_h = [], [], [], []
    for hg in range(H_PER):
        c0 = hg * DHC
        wq_h.append(np.ascontiguousarray(
            np.asarray(Wq[:, c0:c0 + DHC], np.float32).astype(BF)))
        wk_h.append(np.ascontiguousarray(
            np.asarray(Wk[:, c0:c0 + DHC], np.float32).astype(BF)))
        wv_h.append(np.ascontiguousarray(
            np.asarray(Wv[:, c0:c0 + DHC], np.float32).astype(BF)))
        wo_h.append(np.ascontiguousarray(
            np.asarray(Wo[c0:c0 + DHC, :], np.float32).astype(BF)))
    maps = []
    for core in range(N_CORES):
        b, hg = core // H_PER, core % H_PER
        maps.append({
            "xT": xTb[b], "cT": cTb[b], "msk": mb[b],
            "wq": wq_h[hg], "wk": wk_h[hg], "wv": wv_h[hg], "wo": wo_h[hg],
        })
    return maps


def _gather(results, bo):
    out = np.zeros((B, N, C), dtype=np.float32)
    for core in range(N_CORES):
        out[core // H_PER] += np.asarray(results[core]["y"],
                                         dtype=np.float32)
    out += np.asarray(bo, dtype=np.float32)
    return out


def kernel(x, context, mask, Wq, Wk, Wv, Wo, bo, **extra_kwargs):
    if "nc" not in _CACHE:
        _CACHE["nc"] = _build()
    nc = _CACHE["nc"]
    maps = _in_maps(x, context, mask, Wq, Wk, Wv, Wo)
    res = run_bass_kernel_spmd(nc, maps, core_ids=list(range(N_CORES)),
                               **extra_kwargs)
    out = _gather(res.results, bo)
    if extra_kwargs:
        _CACHE["last_result"] = res
    return out
